# revision 44
# baseline (speedup 1.0000x reference)
"""BipartiteGConv Trainium2 kernel (8 NeuronCores, node-range sharding).

Math (see reference):
  rhs = input @ Wi + bi            [N_IN, D]
  lhs = other @ Wo                 [N_OT, D]
  msg = lrelu(rhs[rj] + lhs[lj] + w*We) per edge
  S   = segment_sum(msg, rj)       [N_IN, D]
  out = concat([S @ Wf + bf, input]) @ Wout + bout
      = S @ (Wf@W1) + counts x (bf@W1) + input @ W2 + bout   (W1=Wout[:D], W2=Wout[D:])

Sharding: node ranges of 12500 rj across 8 cores; each core owns all edges
targeting its range.  Edge slots sorted by rj-window of 128 nodes, padded
per window to 128-multiples, uniform across cores (SPMD).

Per 128-edge tile:
  - lhs rows fetched by SWDGE dma_gather from a pair-packed bf16 table in
    DRAM ([25000, 256B] rows holding nodes 2j / 2j+1; idx = lj>>1, fits
    int16 with no A/B split).  Within each window, edges sorted even-lj
    first so the per-edge half-select is 1-2 identity matmuls on
    partition sub-ranges per tile.
  - rhs rows NOT dma-gathered: one-hot matmul against the per-window rhs
    table held in SBUF (edges in a tile share a 128-node window).
  - onehot built on DVE in one batched is_equal per 8 tiles (broadcast
    APs); transposed per-tile on the PE for the rhs gather.
  - message assembled in PSUM by PE matmuls (block-diag mm for w*We,
    identity-mms for lhs halves, onehotT-mm for rhs rows); Lrelu on ACT;
    scatter-sum via onehot matmul accumulated per window in PSUM.
"""
import sys
sys.path.insert(0, "/opt/trn_rl_repo")
import numpy as np
import ml_dtypes

import os

N_IN, N_OT, E, D = 100000, 50000, 1000000, 64
NC = 8
NPC = N_IN // NC            # nodes per core
W = 128                     # window size (nodes)
NW = (NPC + W - 1) // W     # rj windows per core
NWO = (N_OT + W - 1) // W   # lhs table build windows
BLK = int(os.environ.get("KBLK", 1024))  # gather tokens per instruction
GSZ = 8                     # tiles per onehot group
PADV = 999.0                # rjl value for pad slots (onehot -> 0)
KNOTP = bool(os.environ.get("KNOTP"))    # bisect: no partition-offset mms
KNOBC = bool(os.environ.get("KNOBC"))    # bisect: no broadcast-AP oh build
KSP = os.environ.get("KSP")              # override gather single_packet


def _wrap16(a):
    # token i -> [i % 16, i // 16], replicated to 128 partitions
    n = a.shape[0]
    assert n % 16 == 0
    return np.tile(a.reshape(n // 16, 16).T, (8, 1)).copy()


def kernel(input, other, rj, lj, weights, Wi, bi, Wo, We, Wf, bf, Wout, bout):
    import concourse.bass as bass
    import concourse.bacc as bacc
    import concourse.mybir as mybir
    import concourse.tile as tile
    from concourse.bass_utils import run_bass_kernel_spmd
    from contextlib import ExitStack

    input = np.asarray(input, np.float32)
    other = np.asarray(other, np.float32)
    rj = np.asarray(rj).astype(np.int64)
    lj = np.asarray(lj).astype(np.int64)
    weights = np.asarray(weights, np.float32).reshape(-1)
    Wi = np.asarray(Wi, np.float32); bi = np.asarray(bi, np.float32)
    Wo = np.asarray(Wo, np.float32); We = np.asarray(We, np.float32).reshape(-1)
    Wf = np.asarray(Wf, np.float32); bf = np.asarray(bf, np.float32)
    Wout = np.asarray(Wout, np.float32); bout = np.asarray(bout, np.float32)

    bf16 = ml_dtypes.bfloat16

    # ---------------- host index prep (per core) ----------------
    core_of = rj // NPC
    order0 = np.argsort(core_of, kind="stable")
    core_data = []
    for c in range(NC):
        lo = np.searchsorted(core_of, c, side="left", sorter=order0)
        hi = np.searchsorted(core_of, c, side="right", sorter=order0)
        sel = order0[lo:hi]
        rjl_all = rj[sel] - c * NPC
        win = rjl_all // W
        par = (lj[sel] & 1).astype(np.int64)
        key = win * 2 + par          # window-major, even-lj first
        o2 = np.argsort(key, kind="stable")
        core_data.append((sel[o2], (rjl_all % W)[o2], key[o2]))
    TPB = BLK // 128                    # tiles per gather block

    # ---- uniform layout: within each window, even-lj section padded to
    # the max-over-cores even count (so the identity-mm half-select runs
    # are SPMD-uniform), odd section likewise; window padded to a tile
    # multiple.
    cnt_eo = np.zeros((NC, NW, 2), np.int64)
    for c in range(NC):
        sel, rjl_loc, key = core_data[c]
        k2 = np.bincount(key, minlength=2 * NW)
        cnt_eo[c] = k2.reshape(NW, 2)
    # uniform even count per window, rounded to 64 so the mixed tile's
    # half-select split lands on a legal PE base partition (0/64)
    EMAX = (cnt_eo[:, :, 0].max(axis=0) + 63) // 64 * 64
    OMAX = cnt_eo[:, :, 1].max(axis=0)
    WTOT = EMAX + OMAX
    TW = (WTOT + 127) // 128
    assert TW.min() >= 1  # every window closes -> every y row written
    T0 = int(TW.sum())
    padT = (-T0) % TPB
    sched = []
    for w in range(NW):
        sched += [int(w)] * int(TW[w])
    sched += [-1] * padT
    T = len(sched)
    S = T * 128
    NBLK = S // BLK
    NG = T // GSZ
    base_of = {}
    p = 0
    for w in sched:
        if w >= 0 and w not in base_of:
            base_of[w] = p
        p += 128

    lhs_idx = np.zeros((NC, S), np.int16)
    rjl_grid = np.full((NC, S), PADV, np.float32)
    w_grid = np.zeros((NC, S), np.float32)
    counts = np.zeros((NC, NPC), np.float32)
    for c in range(NC):
        sel, rjl_loc, key = core_data[c]
        counts[c] = np.bincount(rj[sel] - c * NPC, minlength=NPC)
        for w in range(NW):
            elo = np.searchsorted(key, 2 * w, side="left")
            ehi = np.searchsorted(key, 2 * w, side="right")
            ohi = np.searchsorted(key, 2 * w + 1, side="right")
            base = base_of[w]
            ne = ehi - elo
            no = ohi - ehi
            if ne:
                idxs = np.arange(base, base + ne)
                ee = sel[elo:ehi]
                lhs_idx[c, idxs] = (lj[ee] >> 1).astype(np.int16)
                rjl_grid[c, idxs] = rjl_loc[elo:ehi].astype(np.float32)
                w_grid[c, idxs] = weights[ee]
            if no:
                ob = base + int(EMAX[w])
                idxs = np.arange(ob, ob + no)
                ee = sel[ehi:ohi]
                lhs_idx[c, idxs] = (lj[ee] >> 1).astype(np.int16)
                rjl_grid[c, idxs] = rjl_loc[ehi:ohi].astype(np.float32)
                w_grid[c, idxs] = weights[ee]

    # per-tile identity-mm runs (p0, p1, half), uniform across cores
    runs = []
    for t in range(T):
        w = sched[t]
        if w < 0:
            runs.append([])
            continue
        j = t - base_of[w] // 128
        k = int(np.clip(EMAX[w] - 128 * j, 0, 128))
        if KNOTP:
            # crash-bisect mode: single full-range mm (numerically wrong
            # for odd-lj edges in mixed tiles)
            runs.append([(0, 128, 0 if k >= 64 else 1)])
            continue
        r = []
        if k > 0:
            r.append((0, k, 0))
        if k < 128:
            r.append((k, 128, 1))
        runs.append(r)

    # slot s -> (p, t) = (s % 128, s // 128) [dma_gather token layout]
    def grid_pt(a, dt_):
        return np.ascontiguousarray(a.reshape(T, 128).T).astype(dt_)

    # rjl in slot-row layout for the ohT broadcast-mm: half-group hg (512
    # slots) lives at [hg % 128, (hg // 128)*512 : ...+512]
    NH = S // 512                       # half-groups (512 slots each)
    HGC = 4                             # half-groups per streamed chunk
    NHP = (NH + HGC - 1) // HGC * HGC

    def rjlr_of(c):
        a = np.zeros((1, NHP * 512), np.float32)
        a[0, :S] = rjl_grid[c]
        return a.astype(bf16)

    # ---------------- build bass kernel ----------------
    dt = mybir.dt
    nc = bacc.Bacc("TRN2", target_bir_lowering=False, debug=False,
                   num_devices=NC, num_swdge_queues=4)

    inT_ext = nc.dram_tensor("inT", [65, NPC], dt.bfloat16, kind="ExternalInput").ap()
    otT_ext = nc.dram_tensor("otT", [64, N_OT], dt.bfloat16, kind="ExternalInput").ap()
    WiB_ext = nc.dram_tensor("WiB", [65, 64], dt.bfloat16, kind="ExternalInput").ap()
    Wo_ext = nc.dram_tensor("Wo_", [64, 64], dt.bfloat16, kind="ExternalInput").ap()
    M1_ext = nc.dram_tensor("M1_", [64, 64], dt.bfloat16, kind="ExternalInput").ap()
    W2_ext = nc.dram_tensor("W2_", [64, 64], dt.bfloat16, kind="ExternalInput").ap()
    vb_ext = nc.dram_tensor("vb_", [2, 64], dt.bfloat16, kind="ExternalInput").ap()
    cnts_ext = nc.dram_tensor("cnts", [2, NPC], dt.bfloat16, kind="ExternalInput").ap()
    iota_ext = nc.dram_tensor("iot", [128, 128], dt.bfloat16, kind="ExternalInput").ap()
    idb_ext = nc.dram_tensor("idb", [128, 128], dt.bfloat16, kind="ExternalInput").ap()
    webd_ext = nc.dram_tensor("webd", [GSZ, GSZ * 64], dt.bfloat16, kind="ExternalInput").ap()
    wT_ext = nc.dram_tensor("wT", [GSZ, NG * 128], dt.bfloat16, kind="ExternalInput").ap()
    lix_ext = nc.dram_tensor("lix", [128, S // 16], dt.int16, kind="ExternalInput").ap()
    rjl_ext = nc.dram_tensor("rjl", [128, T], dt.bfloat16, kind="ExternalInput").ap()
    rjlr_ext = nc.dram_tensor("rjlr", [1, NHP * 512], dt.bfloat16, kind="ExternalInput").ap()
    pio_ext = nc.dram_tensor("pio", [128, 1], dt.float32, kind="ExternalInput").ap()
    y_ext = nc.dram_tensor("y", [NPC, 64], dt.float32, kind="ExternalOutput").ap()

    ltab = nc.dram_tensor("ltab", [N_OT, 64], dt.bfloat16).ap()
    # pair-packed view for the gather: row j = nodes (2j, 2j+1), 256B
    ltab_pairs = ltab.rearrange("(r a) c -> r (a c)", a=2)

    with tile.TileContext(nc) as tc, ExitStack() as ctx:
        cpool = ctx.enter_context(tc.tile_pool(name="const", bufs=1))
        tabp = ctx.enter_context(tc.tile_pool(name="tab", bufs=3))
        gp = ctx.enter_context(tc.tile_pool(name="gath", bufs=3))
        ohp = ctx.enter_context(tc.tile_pool(name="ohp", bufs=3))
        ohtp = ctx.enter_context(tc.tile_pool(name="ohtp", bufs=3))
        wk = ctx.enter_context(tc.tile_pool(name="work", bufs=3))
        psM = ctx.enter_context(tc.tile_pool(name="psM", bufs=3, space="PSUM"))
        psW = ctx.enter_context(tc.tile_pool(name="psW", bufs=2, space="PSUM"))
        psB = ctx.enter_context(tc.tile_pool(name="psB", bufs=2, space="PSUM"))
        accp = ctx.enter_context(tc.tile_pool(name="acc", bufs=1))

        iota = cpool.tile([128, 128], dt.bfloat16)
        nc.sync.dma_start(out=iota[:], in_=iota_ext[:])
        idb = cpool.tile([128, 128], dt.bfloat16)
        nc.sync.dma_start(out=idb[:], in_=idb_ext[:])
        WiB = cpool.tile([65, 64], dt.bfloat16)
        nc.sync.dma_start(out=WiB[:], in_=WiB_ext[:])
        Wo_t = cpool.tile([64, 64], dt.bfloat16)
        nc.sync.dma_start(out=Wo_t[:], in_=Wo_ext[:])
        webd = cpool.tile([GSZ, GSZ * 64], dt.bfloat16)
        nc.sync.dma_start(out=webd[:], in_=webd_ext[:])
        rjl = cpool.tile([128, T], dt.bfloat16)
        nc.sync.dma_start(out=rjl[:], in_=rjl_ext[:])
        pio = cpool.tile([128, 1], dt.float32)
        nc.sync.dma_start(out=pio[:], in_=pio_ext[:])
        lix = cpool.tile([128, S // 16], dt.int16)
        nc.sync.dma_start(out=lix[:], in_=lix_ext[:])
        ones1 = cpool.tile([1, 128], dt.bfloat16)
        nc.vector.memset(ones1[:], 1.0)
        inTs = cpool.tile([65, NPC], dt.bfloat16)
        nc.sync.dma_start(out=inTs[:], in_=inT_ext[:])
        M1t = cpool.tile([64, 64], dt.bfloat16)
        nc.sync.dma_start(out=M1t[:], in_=M1_ext[:])
        W2t = cpool.tile([64, 64], dt.bfloat16)
        nc.sync.dma_start(out=W2t[:], in_=W2_ext[:])
        vbt = cpool.tile([2, 64], dt.bfloat16)
        nc.sync.dma_start(out=vbt[:], in_=vb_ext[:])
        cntr = cpool.tile([2, NPC], dt.bfloat16)
        nc.sync.dma_start(out=cntr[:], in_=cnts_ext[:])

        rhsg = accp.tile([128, NW, 64], dt.bfloat16)

        # ---- build lhs table in DRAM (bf16 rows; gathered pair-packed) ----
        # batched: 16 windows per otT chunk load / staging store
        WCH = 16
        for c0 in range(0, NWO, WCH):
            c1 = min(NWO, c0 + WCH)
            n0 = c0 * W
            n1 = min(N_OT, c1 * W)
            otc = tabp.tile([64, WCH * W], dt.bfloat16, tag="otc")
            nc.sync.dma_start(out=otc[:, :n1 - n0], in_=otT_ext[:, n0:n1])
            stg = tabp.tile([128, WCH, 64], dt.bfloat16, tag="stg")
            for w in range(c0, c1):
                m = min(N_OT, w * W + W) - w * W
                psg = psM.tile([128, GSZ, 64], dt.float32, tag="pm")
                ps = psg[:, 0, :]
                nc.tensor.matmul(out=ps[:m, :],
                                 lhsT=otc[:, (w - c0) * W:(w - c0) * W + m],
                                 rhs=Wo_t[:], start=True, stop=True)
                if w % 2 == 0:
                    nc.scalar.copy(out=stg[:m, w - c0, :], in_=ps[:m, :])
                else:
                    nc.vector.tensor_copy(out=stg[:m, w - c0, :], in_=ps[:m, :])
            fullw = (n1 - n0) // W
            if fullw:
                nc.sync.dma_start(
                    out=ltab[n0:n0 + fullw * W, :].rearrange(
                        "(w p) c -> p w c", p=W),
                    in_=stg[:, :fullw, :])
            rem = (n1 - n0) - fullw * W
            if rem:
                nc.sync.dma_start(out=ltab[n0 + fullw * W:n1, :],
                                  in_=stg[:rem, fullw, :])

        # ---- build rhs table [128, NW, 64] bf16 in SBUF ----
        for w in range(NW):
            n0 = w * W
            n1 = min(NPC, n0 + W)
            m = n1 - n0
            psg = psM.tile([128, GSZ, 64], dt.float32, tag="pm")
            ps = psg[:, 0, :]
            nc.tensor.matmul(out=ps[:m, :], lhsT=inTs[:, n0:n1],
                             rhs=WiB[:], start=True, stop=True)
            if m < 128:
                nc.vector.memset(rhsg[:, w, :], 0.0)
            nc.scalar.copy(out=rhsg[:m, w, :], in_=ps[:m, :])

        # ---- main loop ----
        # windows close in order; each close computes its output rows
        # inline (S_w^T is directly the lhsT of the output matmul) and
        # stages them for a batched y write every EWC windows.
        EWC = 8
        ob_state = {"ob": None, "base": None}

        def emit_y(ob, c0, c1):
            n0 = c0 * W
            n1 = min(NPC, c1 * W)
            fullw = (n1 - n0) // W
            if fullw:
                nc.sync.dma_start(
                    out=y_ext[n0:n0 + fullw * W, :].rearrange(
                        "(w p) c -> p w c", p=W),
                    in_=ob[:, :fullw, :])
            rem = (n1 - n0) - fullw * W
            if rem:
                nc.sync.dma_start(out=y_ext[n0 + fullw * W:n1, :],
                                  in_=ob[:rem, fullw, :])

        def close_window(wv, ps):
            sa = wk.tile([64, 128], dt.bfloat16, tag="sat")
            nc.vector.tensor_copy(out=sa[:], in_=ps[:])
            n0 = wv * W
            n1 = min(NPC, n0 + W)
            m = n1 - n0
            opsg = psM.tile([128, GSZ, 64], dt.float32, tag="pm")
            ops = opsg[:, 0, :]
            nc.tensor.matmul(out=ops[:m, :], lhsT=sa[:, :m], rhs=M1t[:],
                             start=True, stop=False)
            nc.tensor.matmul(out=ops[:m, :], lhsT=inTs[0:64, n0:n1],
                             rhs=W2t[:], start=False, stop=False)
            nc.tensor.matmul(out=ops[:m, :], lhsT=cntr[:, n0:n1],
                             rhs=vbt[:], start=False, stop=True)
            if wv % EWC == 0:
                ob_state["ob"] = wk.tile([128, EWC, 64], dt.float32, tag="ob", name=f"ob{wv}")
                ob_state["base"] = wv
            nc.scalar.copy(out=ob_state["ob"][:m, wv - ob_state["base"], :],
                           in_=ops[:m, :])
            if wv == NW - 1 or wv - ob_state["base"] == EWC - 1:
                emit_y(ob_state["ob"], ob_state["base"], wv + 1)

        cur = {"w": None, "ps": None, "first": True}

        def flush():
            if cur["ps"] is not None:
                close_window(cur["w"], cur["ps"])
                cur["ps"] = None

        rjc = None
        wtc = None
        WTC = 16                        # groups per streamed wT chunk
        for b in range(NBLK):
            gl = gp.tile([128, TPB, 128], dt.bfloat16, tag="gl")
            nc.gpsimd.dma_gather(gl[:], ltab_pairs,
                                 lix[:, b * (BLK // 16):(b + 1) * (BLK // 16)],
                                 BLK, BLK, 128, queue_num=b % 4,
                                 single_packet=(KSP != "0") if KSP is not None
                                 else True)
            for h in range(TPB // GSZ):
                g = b * (TPB // GSZ) + h
                t0 = g * GSZ
                if (2 * g) % HGC == 0:
                    rjc = tabp.tile([1, HGC * 512], dt.bfloat16, tag="rjc")
                    nc.sync.dma_start(
                        out=rjc[:],
                        in_=rjlr_ext[0:1, 2 * g * 512:(2 * g + HGC) * 512])
                if g % WTC == 0:
                    gw1 = min(NG, g + WTC)
                    wtc = tabp.tile([GSZ, WTC * 128], dt.bfloat16, tag="wtc")
                    nc.sync.dma_start(
                        out=wtc[:, :(gw1 - g) * 128],
                        in_=wT_ext[:, g * 128:gw1 * 128])
                oh = ohp.tile([128, GSZ, 128], dt.bfloat16, tag="oh")
                if KNOBC:
                    for i in range(GSZ):
                        nc.vector.tensor_scalar(
                            out=oh[:, i, :], in0=iota[:],
                            scalar1=rjl[:, t0 + i:t0 + i + 1], scalar2=None,
                            op0=mybir.AluOpType.is_equal)
                else:
                    nc.vector.tensor_tensor(
                        out=oh[:],
                        in0=iota[:].unsqueeze(1).broadcast_to([128, GSZ, 128]),
                        in1=rjl[:, t0:t0 + GSZ].unsqueeze(2).broadcast_to([128, GSZ, 128]),
                        op=mybir.AluOpType.is_equal)
                ohT = ohtp.tile([128, GSZ, 128], dt.bfloat16, tag="ohT")
                for hh in range(2):
                    hg = g * 2 + hh
                    off = (hg % HGC) * 512
                    pb = psB.tile([128, 512], dt.float32, tag="bc")
                    nc.tensor.matmul(
                        out=pb[:],
                        lhsT=ones1[:],
                        rhs=rjc[0:1, off:off + 512],
                        start=True, stop=True)
                    nc.vector.tensor_scalar(
                        out=ohT[:, hh * 4:(hh + 1) * 4, :],
                        in0=pb[:].rearrange("p (a c) -> p a c", a=4),
                        scalar1=pio[:], scalar2=None,
                        op0=mybir.AluOpType.is_equal)

                pm = psM.tile([128, GSZ, 64], dt.float32, tag="pm")
                gof = (g % WTC) * 128
                nc.tensor.matmul(out=pm[:], lhsT=wtc[:, gof:gof + 128],
                                 rhs=webd[:], start=True, stop=False)
                last = None
                for i in range(GSZ):
                    if runs[t0 + i]:
                        last = i
                for i in range(GSZ):
                    for (p0, p1, half) in runs[t0 + i]:
                        nc.tensor.matmul(
                            out=pm[p0:p1, i, :], lhsT=idb[:, p0:p1],
                            rhs=gl[:, h * GSZ + i, half * 64:(half + 1) * 64],
                            start=False, stop=False, skip_group_check=True)
                for i in range(GSZ):
                    if not runs[t0 + i]:
                        continue
                    wv = sched[t0 + i]
                    nc.tensor.matmul(out=pm[:, i, :], lhsT=ohT[:, i, :],
                                     rhs=rhsg[:, wv, :],
                                     start=False, stop=(i == last),
                                     skip_group_check=True)
                mrb = wk.tile([128, GSZ, 64], dt.bfloat16, tag="mrb")
                nc.scalar.activation(out=mrb[:], in_=pm[:],
                                     func=mybir.ActivationFunctionType.Lrelu,
                                     alpha=0.01)
                for i in range(GSZ):
                    if not runs[t0 + i]:
                        continue
                    wv = sched[t0 + i]
                    if cur["w"] != wv:
                        flush()
                        cur["w"] = wv
                        cur["ps"] = psW.tile([64, 128], dt.float32, tag="psw",
                                             name=f"psw{t0 + i}")
                        cur["first"] = True
                    nc.tensor.matmul(out=cur["ps"][:], lhsT=mrb[:, i, :],
                                     rhs=oh[:, i, :],
                                     start=cur["first"], stop=False)
                    cur["first"] = False
        flush()

    nc.compile()

    import os
    if os.environ.get("KDRY"):
        print(f"dry-run OK: T={T} S={S} NBLK={NBLK} NG={NG}")
        return np.zeros((N_IN, 64), np.float32)

    # ---------------- host-side in_maps ----------------
    W1 = Wout[:64]; W2 = Wout[64:]
    M1 = (Wf @ W1).astype(np.float32)
    v1 = (bf @ W1).astype(np.float32)
    vb = np.stack([v1, bout]).astype(bf16)
    iota_np = np.tile(np.arange(128, dtype=np.float32)[None, :], (128, 1)).astype(bf16)
    idb_np = np.eye(128, dtype=np.float32).astype(bf16)
    WiB_np = np.concatenate([Wi, bi[None, :]], 0).astype(bf16)
    webd_np = np.zeros((GSZ, GSZ * 64), np.float32)
    for i in range(GSZ):
        webd_np[i, i * 64:(i + 1) * 64] = We
    webd_np = webd_np.astype(bf16)

    in_maps = []
    for c in range(NC):
        sl = input[c * NPC:(c + 1) * NPC]
        inT = np.concatenate([sl.T, np.ones((1, NPC), np.float32)], 0).astype(bf16)
        wTc = np.ascontiguousarray(
            w_grid[c].reshape(NG, GSZ, 128).transpose(1, 0, 2).reshape(GSZ, NG * 128)
        ).astype(bf16)
        in_maps.append({
            "inT": np.ascontiguousarray(inT),
            "otT": np.ascontiguousarray(other.T).astype(bf16),
            "WiB": WiB_np, "Wo_": Wo.astype(bf16),
            "M1_": M1.astype(bf16), "W2_": W2.astype(bf16), "vb_": vb,
            "cnts": np.stack([counts[c], np.ones(NPC, np.float32)]).astype(bf16),
            "iot": iota_np, "idb": idb_np,
            "webd": webd_np, "wT": wTc,
            "lix": _wrap16(lhs_idx[c]),
            "rjl": grid_pt(rjl_grid[c], bf16),
            "rjlr": rjlr_of(c),
            "pio": np.arange(128, dtype=np.float32).reshape(128, 1),
        })

    import os, tempfile
    global LAST_TMPDIR
    LAST_TMPDIR = tempfile.mkdtemp(prefix="ktrace_")
    res = run_bass_kernel_spmd(nc, in_maps, list(range(NC)),
                               tmpdir=LAST_TMPDIR,
                               trace=bool(os.environ.get("KTRACE")))
    if os.environ.get("KTRACE") and res.exec_time_ns:
        print(f"HW exec time: {res.exec_time_ns} ns")
    out = np.concatenate([res.results[c]["y"] for c in range(NC)], 0)
    return out.astype(np.float32)


# revision 45
# speedup vs baseline: 1.0766x; 1.0766x over previous
"""BipartiteGConv Trainium2 kernel (8 NeuronCores, node-range sharding).

Math (see reference):
  rhs = input @ Wi + bi            [N_IN, D]
  lhs = other @ Wo                 [N_OT, D]
  msg = lrelu(rhs[rj] + lhs[lj] + w*We) per edge
  S   = segment_sum(msg, rj)       [N_IN, D]
  out = concat([S @ Wf + bf, input]) @ Wout + bout
      = S @ (Wf@W1) + counts x (bf@W1) + input @ W2 + bout   (W1=Wout[:D], W2=Wout[D:])

Sharding: node ranges of 12500 rj across 8 cores; each core owns all edges
targeting its range.  Edge slots sorted by rj-window of 128 nodes, padded
per window to 128-multiples, uniform across cores (SPMD).

Per 128-edge tile:
  - lhs rows fetched by SWDGE dma_gather from a pair-packed bf16 table in
    DRAM ([25000, 256B] rows holding nodes 2j / 2j+1; idx = lj>>1, fits
    int16 with no A/B split).  Within each window, edges sorted even-lj
    first so the per-edge half-select is 1-2 identity matmuls on
    partition sub-ranges per tile.
  - rhs rows NOT dma-gathered: one-hot matmul against the per-window rhs
    table held in SBUF (edges in a tile share a 128-node window).
  - onehot built on DVE in one batched is_equal per 8 tiles (broadcast
    APs); transposed per-tile on the PE for the rhs gather.
  - message assembled in PSUM by PE matmuls (block-diag mm for w*We,
    identity-mms for lhs halves, onehotT-mm for rhs rows); Lrelu on ACT;
    scatter-sum via onehot matmul accumulated per window in PSUM.
"""
import sys
sys.path.insert(0, "/opt/trn_rl_repo")
import numpy as np
import ml_dtypes

import os

N_IN, N_OT, E, D = 100000, 50000, 1000000, 64
NC = 8
NPC = N_IN // NC            # nodes per core
W = 128                     # window size (nodes)
NW = (NPC + W - 1) // W     # rj windows per core
NWO = (N_OT + W - 1) // W   # lhs table build windows
BLK = int(os.environ.get("KBLK", 1024))  # gather tokens per instruction
GSZ = 8                     # tiles per onehot group
PADV = 999.0                # rjl value for pad slots (onehot -> 0)
KNOTP = bool(os.environ.get("KNOTP"))    # bisect: no partition-offset mms
KNOBC = bool(os.environ.get("KNOBC"))    # bisect: no broadcast-AP oh build
KSP = os.environ.get("KSP")              # override gather single_packet


def _wrap16(a):
    # token i -> [i % 16, i // 16], replicated to 128 partitions
    n = a.shape[0]
    assert n % 16 == 0
    return np.tile(a.reshape(n // 16, 16).T, (8, 1)).copy()


def kernel(input, other, rj, lj, weights, Wi, bi, Wo, We, Wf, bf, Wout, bout):
    import concourse.bass as bass
    import concourse.bacc as bacc
    import concourse.mybir as mybir
    import concourse.tile as tile
    from concourse.bass_utils import run_bass_kernel_spmd
    from contextlib import ExitStack

    input = np.asarray(input, np.float32)
    other = np.asarray(other, np.float32)
    rj = np.asarray(rj).astype(np.int64)
    lj = np.asarray(lj).astype(np.int64)
    weights = np.asarray(weights, np.float32).reshape(-1)
    Wi = np.asarray(Wi, np.float32); bi = np.asarray(bi, np.float32)
    Wo = np.asarray(Wo, np.float32); We = np.asarray(We, np.float32).reshape(-1)
    Wf = np.asarray(Wf, np.float32); bf = np.asarray(bf, np.float32)
    Wout = np.asarray(Wout, np.float32); bout = np.asarray(bout, np.float32)

    bf16 = ml_dtypes.bfloat16

    # ---------------- host index prep (per core) ----------------
    core_of = rj // NPC
    order0 = np.argsort(core_of, kind="stable")
    core_data = []
    for c in range(NC):
        lo = np.searchsorted(core_of, c, side="left", sorter=order0)
        hi = np.searchsorted(core_of, c, side="right", sorter=order0)
        sel = order0[lo:hi]
        rjl_all = rj[sel] - c * NPC
        win = rjl_all // W
        par = (lj[sel] & 1).astype(np.int64)
        key = win * 2 + par          # window-major, even-lj first
        o2 = np.argsort(key, kind="stable")
        core_data.append((sel[o2], (rjl_all % W)[o2], key[o2]))
    TPB = BLK // 128                    # tiles per gather block

    # ---- uniform layout: within each window, even-lj section padded to
    # the max-over-cores even count (so the identity-mm half-select runs
    # are SPMD-uniform), odd section likewise; window padded to a tile
    # multiple.
    cnt_eo = np.zeros((NC, NW, 2), np.int64)
    for c in range(NC):
        sel, rjl_loc, key = core_data[c]
        k2 = np.bincount(key, minlength=2 * NW)
        cnt_eo[c] = k2.reshape(NW, 2)
    # uniform even count per window, rounded to 64 so the mixed tile's
    # half-select split lands on a legal PE base partition (0/64)
    EMAX = (cnt_eo[:, :, 0].max(axis=0) + 63) // 64 * 64
    OMAX = cnt_eo[:, :, 1].max(axis=0)
    WTOT = EMAX + OMAX
    TW = (WTOT + 127) // 128
    assert TW.min() >= 1  # every window closes -> every y row written
    T0 = int(TW.sum())
    padT = (-T0) % TPB
    sched = []
    for w in range(NW):
        sched += [int(w)] * int(TW[w])
    sched += [-1] * padT
    T = len(sched)
    S = T * 128
    NBLK = S // BLK
    NG = T // GSZ
    base_of = {}
    p = 0
    for w in sched:
        if w >= 0 and w not in base_of:
            base_of[w] = p
        p += 128

    lhs_idx = np.zeros((NC, S), np.int16)
    rjl_grid = np.full((NC, S), PADV, np.float32)
    w_grid = np.zeros((NC, S), np.float32)
    counts = np.zeros((NC, NPC), np.float32)
    for c in range(NC):
        sel, rjl_loc, key = core_data[c]
        counts[c] = np.bincount(rj[sel] - c * NPC, minlength=NPC)
        for w in range(NW):
            elo = np.searchsorted(key, 2 * w, side="left")
            ehi = np.searchsorted(key, 2 * w, side="right")
            ohi = np.searchsorted(key, 2 * w + 1, side="right")
            base = base_of[w]
            ne = ehi - elo
            no = ohi - ehi
            if ne:
                idxs = np.arange(base, base + ne)
                ee = sel[elo:ehi]
                lhs_idx[c, idxs] = (lj[ee] >> 1).astype(np.int16)
                rjl_grid[c, idxs] = rjl_loc[elo:ehi].astype(np.float32)
                w_grid[c, idxs] = weights[ee]
            if no:
                ob = base + int(EMAX[w])
                idxs = np.arange(ob, ob + no)
                ee = sel[ehi:ohi]
                lhs_idx[c, idxs] = (lj[ee] >> 1).astype(np.int16)
                rjl_grid[c, idxs] = rjl_loc[ehi:ohi].astype(np.float32)
                w_grid[c, idxs] = weights[ee]

    # per-tile identity-mm runs (p0, p1, half), uniform across cores
    runs = []
    for t in range(T):
        w = sched[t]
        if w < 0:
            runs.append([])
            continue
        j = t - base_of[w] // 128
        k = int(np.clip(EMAX[w] - 128 * j, 0, 128))
        if KNOTP:
            # crash-bisect mode: single full-range mm (numerically wrong
            # for odd-lj edges in mixed tiles)
            runs.append([(0, 128, 0 if k >= 64 else 1)])
            continue
        r = []
        if k > 0:
            r.append((0, k, 0))
        if k < 128:
            r.append((k, 128, 1))
        runs.append(r)

    # slot s -> (p, t) = (s % 128, s // 128) [dma_gather token layout]
    def grid_pt(a, dt_):
        return np.ascontiguousarray(a.reshape(T, 128).T).astype(dt_)

    # rjl in slot-row layout for the ohT broadcast-mm: half-group hg (512
    # slots) lives at [hg % 128, (hg // 128)*512 : ...+512]
    NH = S // 512                       # half-groups (512 slots each)
    HGC = 4                             # half-groups per streamed chunk
    NHP = (NH + HGC - 1) // HGC * HGC

    def rjlr_of(c):
        a = np.zeros((1, NHP * 512), np.float32)
        a[0, :S] = rjl_grid[c]
        return a.astype(bf16)

    # ---------------- build bass kernel ----------------
    dt = mybir.dt
    nc = bacc.Bacc("TRN2", target_bir_lowering=False, debug=False,
                   num_devices=NC, num_swdge_queues=4)

    inT_ext = nc.dram_tensor("inT", [65, NPC], dt.bfloat16, kind="ExternalInput").ap()
    otT_ext = nc.dram_tensor("otT", [64, N_OT], dt.bfloat16, kind="ExternalInput").ap()
    WiB_ext = nc.dram_tensor("WiB", [65, 64], dt.bfloat16, kind="ExternalInput").ap()
    Wo_ext = nc.dram_tensor("Wo_", [64, 64], dt.bfloat16, kind="ExternalInput").ap()
    M1_ext = nc.dram_tensor("M1_", [64, 64], dt.bfloat16, kind="ExternalInput").ap()
    W2_ext = nc.dram_tensor("W2_", [64, 64], dt.bfloat16, kind="ExternalInput").ap()
    vb_ext = nc.dram_tensor("vb_", [2, 64], dt.bfloat16, kind="ExternalInput").ap()
    cnts_ext = nc.dram_tensor("cnts", [2, NPC], dt.bfloat16, kind="ExternalInput").ap()
    iota_ext = nc.dram_tensor("iot", [128, 128], dt.bfloat16, kind="ExternalInput").ap()
    idb_ext = nc.dram_tensor("idb", [128, 128], dt.bfloat16, kind="ExternalInput").ap()
    webd_ext = nc.dram_tensor("webd", [GSZ, GSZ * 64], dt.bfloat16, kind="ExternalInput").ap()
    wT_ext = nc.dram_tensor("wT", [GSZ, NG * 128], dt.bfloat16, kind="ExternalInput").ap()
    lix_ext = nc.dram_tensor("lix", [128, S // 16], dt.int16, kind="ExternalInput").ap()
    rjl_ext = nc.dram_tensor("rjl", [128, T], dt.bfloat16, kind="ExternalInput").ap()
    rjlr_ext = nc.dram_tensor("rjlr", [1, NHP * 512], dt.bfloat16, kind="ExternalInput").ap()
    pio_ext = nc.dram_tensor("pio", [128, 1], dt.float32, kind="ExternalInput").ap()
    y_ext = nc.dram_tensor("y", [NPC, 64], dt.float32, kind="ExternalOutput").ap()

    ltab = nc.dram_tensor("ltab", [N_OT, 64], dt.bfloat16).ap()
    # pair-packed view for the gather: row j = nodes (2j, 2j+1), 256B
    ltab_pairs = ltab.rearrange("(r a) c -> r (a c)", a=2)

    with tile.TileContext(nc) as tc, ExitStack() as ctx:
        cpool = ctx.enter_context(tc.tile_pool(name="const", bufs=1))
        tabp = ctx.enter_context(tc.tile_pool(name="tab", bufs=3))
        gp = ctx.enter_context(tc.tile_pool(name="gath", bufs=3))
        ohp = ctx.enter_context(tc.tile_pool(name="ohp", bufs=3))
        ohtp = ctx.enter_context(tc.tile_pool(name="ohtp", bufs=3))
        wk = ctx.enter_context(tc.tile_pool(name="work", bufs=3))
        psM = ctx.enter_context(tc.tile_pool(name="psM", bufs=3, space="PSUM"))
        psW = ctx.enter_context(tc.tile_pool(name="psW", bufs=2, space="PSUM"))
        psB = ctx.enter_context(tc.tile_pool(name="psB", bufs=2, space="PSUM"))
        accp = ctx.enter_context(tc.tile_pool(name="acc", bufs=1))

        iota = cpool.tile([128, 128], dt.bfloat16)
        nc.sync.dma_start(out=iota[:], in_=iota_ext[:])
        idb = cpool.tile([128, 128], dt.bfloat16)
        nc.sync.dma_start(out=idb[:], in_=idb_ext[:])
        WiB = cpool.tile([65, 64], dt.bfloat16)
        nc.sync.dma_start(out=WiB[:], in_=WiB_ext[:])
        Wo_t = cpool.tile([64, 64], dt.bfloat16)
        nc.sync.dma_start(out=Wo_t[:], in_=Wo_ext[:])
        webd = cpool.tile([GSZ, GSZ * 64], dt.bfloat16)
        nc.sync.dma_start(out=webd[:], in_=webd_ext[:])
        rjl = cpool.tile([128, T], dt.bfloat16)
        nc.sync.dma_start(out=rjl[:], in_=rjl_ext[:])
        pio = cpool.tile([128, 1], dt.float32)
        nc.sync.dma_start(out=pio[:], in_=pio_ext[:])
        lix = cpool.tile([128, S // 16], dt.int16)
        nc.sync.dma_start(out=lix[:], in_=lix_ext[:])
        ones1 = cpool.tile([1, 128], dt.bfloat16)
        nc.vector.memset(ones1[:], 1.0)
        inTs = cpool.tile([65, NPC], dt.bfloat16)
        nc.sync.dma_start(out=inTs[:], in_=inT_ext[:])
        M1t = cpool.tile([64, 64], dt.bfloat16)
        nc.sync.dma_start(out=M1t[:], in_=M1_ext[:])
        W2t = cpool.tile([64, 64], dt.bfloat16)
        nc.sync.dma_start(out=W2t[:], in_=W2_ext[:])
        vbt = cpool.tile([2, 64], dt.bfloat16)
        nc.sync.dma_start(out=vbt[:], in_=vb_ext[:])
        cntr = cpool.tile([2, NPC], dt.bfloat16)
        nc.sync.dma_start(out=cntr[:], in_=cnts_ext[:])

        rhsg = accp.tile([128, NW, 64], dt.bfloat16)

        # ---- build lhs table in DRAM (bf16 rows; gathered pair-packed) ----
        # batched: 32 windows per otT chunk load / staging store, 2 windows
        # per PSUM tile so copies amortize
        WCH = 32
        for c0 in range(0, NWO, WCH):
            c1 = min(NWO, c0 + WCH)
            n0 = c0 * W
            n1 = min(N_OT, c1 * W)
            otc = tabp.tile([64, WCH * W], dt.bfloat16, tag="otc")
            nc.sync.dma_start(out=otc[:, :n1 - n0], in_=otT_ext[:, n0:n1])
            stg = tabp.tile([128, WCH, 64], dt.bfloat16, tag="stg")
            for w0 in range(c0, c1, 2):
                psg = psM.tile([128, GSZ, 64], dt.float32, tag="pm")
                nw2 = min(2, c1 - w0)
                mm = 0
                for j in range(nw2):
                    w = w0 + j
                    m = min(N_OT, w * W + W) - w * W
                    mm = max(mm, m)
                    nc.tensor.matmul(out=psg[:m, j, :],
                                     lhsT=otc[:, (w - c0) * W:(w - c0) * W + m],
                                     rhs=Wo_t[:], start=True, stop=True)
                if (w0 // 2) % 2 == 0:
                    nc.scalar.copy(out=stg[:mm, w0 - c0:w0 - c0 + nw2, :],
                                   in_=psg[:mm, 0:nw2, :])
                else:
                    nc.vector.tensor_copy(out=stg[:mm, w0 - c0:w0 - c0 + nw2, :],
                                          in_=psg[:mm, 0:nw2, :])
            fullw = (n1 - n0) // W
            if fullw:
                nc.sync.dma_start(
                    out=ltab[n0:n0 + fullw * W, :].rearrange(
                        "(w p) c -> p w c", p=W),
                    in_=stg[:, :fullw, :])
            rem = (n1 - n0) - fullw * W
            if rem:
                nc.sync.dma_start(out=ltab[n0 + fullw * W:n1, :],
                                  in_=stg[:rem, fullw, :])

        # ---- build rhs table [128, NW, 64] bf16 in SBUF ----
        for w in range(NW):
            n0 = w * W
            n1 = min(NPC, n0 + W)
            m = n1 - n0
            psg = psM.tile([128, GSZ, 64], dt.float32, tag="pm")
            ps = psg[:, 0, :]
            nc.tensor.matmul(out=ps[:m, :], lhsT=inTs[:, n0:n1],
                             rhs=WiB[:], start=True, stop=True)
            if m < 128:
                nc.vector.memset(rhsg[:, w, :], 0.0)
            nc.scalar.copy(out=rhsg[:m, w, :], in_=ps[:m, :])

        # ---- main loop ----
        # windows close in order; each close computes its output rows
        # inline (S_w^T is directly the lhsT of the output matmul) and
        # stages them for a batched y write every EWC windows.
        EWC = 8
        ob_state = {"ob": None, "base": None}

        def emit_y(ob, c0, c1):
            n0 = c0 * W
            n1 = min(NPC, c1 * W)
            fullw = (n1 - n0) // W
            if fullw:
                nc.sync.dma_start(
                    out=y_ext[n0:n0 + fullw * W, :].rearrange(
                        "(w p) c -> p w c", p=W),
                    in_=ob[:, :fullw, :])
            rem = (n1 - n0) - fullw * W
            if rem:
                nc.sync.dma_start(out=y_ext[n0 + fullw * W:n1, :],
                                  in_=ob[:rem, fullw, :])

        def close_window(wv, ps):
            sa = wk.tile([64, 128], dt.bfloat16, tag="sat")
            nc.vector.tensor_copy(out=sa[:], in_=ps[:])
            n0 = wv * W
            n1 = min(NPC, n0 + W)
            m = n1 - n0
            opsg = psM.tile([128, GSZ, 64], dt.float32, tag="pm")
            ops = opsg[:, 0, :]
            nc.tensor.matmul(out=ops[:m, :], lhsT=sa[:, :m], rhs=M1t[:],
                             start=True, stop=False)
            nc.tensor.matmul(out=ops[:m, :], lhsT=inTs[0:64, n0:n1],
                             rhs=W2t[:], start=False, stop=False)
            nc.tensor.matmul(out=ops[:m, :], lhsT=cntr[:, n0:n1],
                             rhs=vbt[:], start=False, stop=True)
            if wv % EWC == 0:
                ob_state["ob"] = wk.tile([128, EWC, 64], dt.float32, tag="ob", name=f"ob{wv}")
                ob_state["base"] = wv
            nc.scalar.copy(out=ob_state["ob"][:m, wv - ob_state["base"], :],
                           in_=ops[:m, :])
            if wv == NW - 1 or wv - ob_state["base"] == EWC - 1:
                emit_y(ob_state["ob"], ob_state["base"], wv + 1)

        cur = {"w": None, "ps": None, "first": True}

        def flush():
            if cur["ps"] is not None:
                close_window(cur["w"], cur["ps"])
                cur["ps"] = None

        rjc = None
        wtc = None
        WTC = 16                        # groups per streamed wT chunk
        for b in range(NBLK):
            gl = gp.tile([128, TPB, 128], dt.bfloat16, tag="gl")
            nc.gpsimd.dma_gather(gl[:], ltab_pairs,
                                 lix[:, b * (BLK // 16):(b + 1) * (BLK // 16)],
                                 BLK, BLK, 128, queue_num=b % 4,
                                 single_packet=(KSP != "0") if KSP is not None
                                 else True)
            for h in range(TPB // GSZ):
                g = b * (TPB // GSZ) + h
                t0 = g * GSZ
                if (2 * g) % HGC == 0:
                    rjc = tabp.tile([1, HGC * 512], dt.bfloat16, tag="rjc")
                    nc.sync.dma_start(
                        out=rjc[:],
                        in_=rjlr_ext[0:1, 2 * g * 512:(2 * g + HGC) * 512])
                if g % WTC == 0:
                    gw1 = min(NG, g + WTC)
                    wtc = tabp.tile([GSZ, WTC * 128], dt.bfloat16, tag="wtc")
                    nc.sync.dma_start(
                        out=wtc[:, :(gw1 - g) * 128],
                        in_=wT_ext[:, g * 128:gw1 * 128])
                oh = ohp.tile([128, GSZ, 128], dt.bfloat16, tag="oh")
                if KNOBC:
                    for i in range(GSZ):
                        nc.vector.tensor_scalar(
                            out=oh[:, i, :], in0=iota[:],
                            scalar1=rjl[:, t0 + i:t0 + i + 1], scalar2=None,
                            op0=mybir.AluOpType.is_equal)
                else:
                    nc.vector.tensor_tensor(
                        out=oh[:],
                        in0=iota[:].unsqueeze(1).broadcast_to([128, GSZ, 128]),
                        in1=rjl[:, t0:t0 + GSZ].unsqueeze(2).broadcast_to([128, GSZ, 128]),
                        op=mybir.AluOpType.is_equal)
                ohT = ohtp.tile([128, GSZ, 128], dt.bfloat16, tag="ohT")
                for hh in range(2):
                    hg = g * 2 + hh
                    off = (hg % HGC) * 512
                    pb = psB.tile([128, 512], dt.float32, tag="bc")
                    nc.tensor.matmul(
                        out=pb[:],
                        lhsT=ones1[:],
                        rhs=rjc[0:1, off:off + 512],
                        start=True, stop=True)
                    nc.vector.tensor_scalar(
                        out=ohT[:, hh * 4:(hh + 1) * 4, :],
                        in0=pb[:].rearrange("p (a c) -> p a c", a=4),
                        scalar1=pio[:], scalar2=None,
                        op0=mybir.AluOpType.is_equal)

                pm = psM.tile([128, GSZ, 64], dt.float32, tag="pm")
                gof = (g % WTC) * 128
                nc.tensor.matmul(out=pm[:], lhsT=wtc[:, gof:gof + 128],
                                 rhs=webd[:], start=True, stop=False)
                last = None
                for i in range(GSZ):
                    if runs[t0 + i]:
                        last = i
                for i in range(GSZ):
                    for (p0, p1, half) in runs[t0 + i]:
                        nc.tensor.matmul(
                            out=pm[p0:p1, i, :], lhsT=idb[:, p0:p1],
                            rhs=gl[:, h * GSZ + i, half * 64:(half + 1) * 64],
                            start=False, stop=False, skip_group_check=True)
                for i in range(GSZ):
                    if not runs[t0 + i]:
                        continue
                    wv = sched[t0 + i]
                    nc.tensor.matmul(out=pm[:, i, :], lhsT=ohT[:, i, :],
                                     rhs=rhsg[:, wv, :],
                                     start=False, stop=(i == last),
                                     skip_group_check=True)
                mrb = wk.tile([128, GSZ, 64], dt.bfloat16, tag="mrb")
                nc.scalar.activation(out=mrb[:], in_=pm[:],
                                     func=mybir.ActivationFunctionType.Lrelu,
                                     alpha=0.01)
                for i in range(GSZ):
                    if not runs[t0 + i]:
                        continue
                    wv = sched[t0 + i]
                    if cur["w"] != wv:
                        flush()
                        cur["w"] = wv
                        cur["ps"] = psW.tile([64, 128], dt.float32, tag="psw",
                                             name=f"psw{t0 + i}")
                        cur["first"] = True
                    nc.tensor.matmul(out=cur["ps"][:], lhsT=mrb[:, i, :],
                                     rhs=oh[:, i, :],
                                     start=cur["first"], stop=False)
                    cur["first"] = False
        flush()

    nc.compile()

    import os
    if os.environ.get("KDRY"):
        print(f"dry-run OK: T={T} S={S} NBLK={NBLK} NG={NG}")
        return np.zeros((N_IN, 64), np.float32)

    # ---------------- host-side in_maps ----------------
    W1 = Wout[:64]; W2 = Wout[64:]
    M1 = (Wf @ W1).astype(np.float32)
    v1 = (bf @ W1).astype(np.float32)
    vb = np.stack([v1, bout]).astype(bf16)
    iota_np = np.tile(np.arange(128, dtype=np.float32)[None, :], (128, 1)).astype(bf16)
    idb_np = np.eye(128, dtype=np.float32).astype(bf16)
    WiB_np = np.concatenate([Wi, bi[None, :]], 0).astype(bf16)
    webd_np = np.zeros((GSZ, GSZ * 64), np.float32)
    for i in range(GSZ):
        webd_np[i, i * 64:(i + 1) * 64] = We
    webd_np = webd_np.astype(bf16)

    in_maps = []
    for c in range(NC):
        sl = input[c * NPC:(c + 1) * NPC]
        inT = np.concatenate([sl.T, np.ones((1, NPC), np.float32)], 0).astype(bf16)
        wTc = np.ascontiguousarray(
            w_grid[c].reshape(NG, GSZ, 128).transpose(1, 0, 2).reshape(GSZ, NG * 128)
        ).astype(bf16)
        in_maps.append({
            "inT": np.ascontiguousarray(inT),
            "otT": np.ascontiguousarray(other.T).astype(bf16),
            "WiB": WiB_np, "Wo_": Wo.astype(bf16),
            "M1_": M1.astype(bf16), "W2_": W2.astype(bf16), "vb_": vb,
            "cnts": np.stack([counts[c], np.ones(NPC, np.float32)]).astype(bf16),
            "iot": iota_np, "idb": idb_np,
            "webd": webd_np, "wT": wTc,
            "lix": _wrap16(lhs_idx[c]),
            "rjl": grid_pt(rjl_grid[c], bf16),
            "rjlr": rjlr_of(c),
            "pio": np.arange(128, dtype=np.float32).reshape(128, 1),
        })

    import os, tempfile
    global LAST_TMPDIR
    LAST_TMPDIR = tempfile.mkdtemp(prefix="ktrace_")
    res = run_bass_kernel_spmd(nc, in_maps, list(range(NC)),
                               tmpdir=LAST_TMPDIR,
                               trace=bool(os.environ.get("KTRACE")))
    if os.environ.get("KTRACE") and res.exec_time_ns:
        print(f"HW exec time: {res.exec_time_ns} ns")
    out = np.concatenate([res.results[c]["y"] for c in range(NC)], 0)
    return out.astype(np.float32)
